# revision 1
# baseline (speedup 1.0000x reference)
"""SAGEConv-style GNN message passing on 8 Trainium2 NeuronCores.

out = (mean_{j in N(i)} x_j) @ W_l + b_l + x_i @ W_r
with N(i) defined by edge_index ([2, E]: src=row0, dst=row1), mean over
in-edges (segment mean by dst), N=100000 nodes, E=6400000 edges.

Distribution strategy (chosen; spec hint allows any): shard by DESTINATION
node range — core c owns nodes [c*12500, (c+1)*12500) and receives exactly
the edges targeting its range, laid out dst-major.  Each core then computes
its 12500-row output slice independently — no collective needed.

Host-side work is limited to sharding/layout: slicing edges to cores,
ordering each core's shard by destination (edge order is semantically
irrelevant to segment_sum), and packing edge lists into fixed-width
partition rows (a CSR-style layout conversion).  All arithmetic on values
(gather of x rows, segment sums, counts, means, both matmuls, bias) runs
on device.

Device algorithm per core:
  1. Bulk-DMA src/dst index rows (int32) per chunk [128, TC].
  2. Indirect-DMA gather of x_aug rows (12 f32: 10 features, ones col,
     pad) per edge slot -> msgs [128, TC, 12].
  3. keep-mask from dst equality with left neighbor; per-feature
     segmented inclusive cumsum via tensor_tensor_scan
     (state = keep*state + msg).  Rows start at node boundaries, so no
     cross-row carries exist.
  4. Store scan results (P) to DRAM scratch.
  5. Gather P rows at each node's closing edge position -> per-node
     [sum(x_j), count].
  6. Epilogue: out = (S/max(cnt,1)) @ W_l + b_l + x_i @ W_r via
     per-column scalar_tensor_tensor ops.

Performance note: the per-edge gather is the bottleneck.  The hardware
contract for indirect DMA is one offset per partition per instruction
(128 rows / instruction), so the ~917k gathers per core take ~7k Pool
instructions; everything else (scan, stores, epilogue) overlaps under
them.  Bulk multi-offset indirect forms and the MoE dma_gather path were
tested and do not work correctly in this environment (see repo notes).
"""

import numpy as np

import concourse.bass as bass
import concourse.tile as tile
from concourse import bacc, mybir

# ---------------------------------------------------------------- config
N_NODES = 100000
N_EDGES = 6400000
IN_DIM = 10
HIDDEN = 16
N_CORES = 8

F = 12          # x_aug row width: 10 features + ones + pad (48B rows)
TC = 512        # edge slots per partition row
N_CHUNKS = 13   # chunks of [128, TC] edge slots per core
ROWS = 128 * N_CHUNKS          # partition rows per core
E_PACK = ROWS * TC             # packed edge slots per core
NODES_PC = N_NODES // N_CORES  # 12500
NPAD = 12544                   # padded node count per core (=128*98)
NJ = NPAD // 128               # nodes per partition in epilogue (98)
PAD_ROW = N_NODES              # x_aug row of zeros used by pad slots


# ---------------------------------------------------------------- device
def build_program(n_chunks=N_CHUNKS, tc_len=TC, npad=NPAD, n_nodes_aug=N_NODES + 1,
                  num_devices=N_CORES):
    """Build the per-core Bass program (SPMD: same program, per-core data)."""
    P = 128
    nj = npad // P
    e_pack = n_chunks * P * tc_len
    nc = bacc.Bacc("TRN2", target_bir_lowering=False, debug=False,
                   num_devices=num_devices)

    x_aug = nc.dram_tensor("x_aug", [n_nodes_aug, F], mybir.dt.float32,
                           kind="ExternalInput")
    srcs = nc.dram_tensor("srcs", [e_pack], mybir.dt.int32, kind="ExternalInput")
    dsts = nc.dram_tensor("dsts", [e_pack], mybir.dt.int32, kind="ExternalInput")
    close_idx = nc.dram_tensor("close_idx", [npad], mybir.dt.int32,
                               kind="ExternalInput")
    carry_idx = nc.dram_tensor("carry_idx", [npad], mybir.dt.int32,
                               kind="ExternalInput")
    xsh = nc.dram_tensor("xsh", [npad, F], mybir.dt.float32, kind="ExternalInput")
    wrep = nc.dram_tensor("wrep", [P, 2 * IN_DIM * HIDDEN + HIDDEN],
                          mybir.dt.float32, kind="ExternalInput")
    p_all = nc.dram_tensor("p_all", [e_pack, F], mybir.dt.float32, kind="Internal")
    out_d = nc.dram_tensor("out", [npad, HIDDEN], mybir.dt.float32,
                           kind="ExternalOutput")

    with tile.TileContext(nc) as tc:
        with (
            tc.tile_pool(name="idx", bufs=3) as idx_pool,
            tc.tile_pool(name="msg", bufs=3) as msg_pool,
            tc.tile_pool(name="scan", bufs=3) as scan_pool,
            tc.tile_pool(name="epi", bufs=1) as epi_pool,
        ):
            srcs_v = srcs.ap().rearrange("(c p t) -> c p t", p=P, t=tc_len)
            dsts_v = dsts.ap().rearrange("(c p t) -> c p t", p=P, t=tc_len)
            pall_v = p_all.ap().rearrange("(c p t) f -> c p (t f)", p=P, t=tc_len)

            for ch in range(n_chunks):
                # the tail chunk is mostly padding: the host packs its edges
                # into the first tc_len//4 columns only, so its loops shrink.
                cols = tc_len if ch < n_chunks - 1 else tc_len // 4
                src_t = idx_pool.tile([P, tc_len], mybir.dt.int32, tag="src")
                nc.sync.dma_start(src_t[:, :cols], srcs_v[ch][:, :cols])
                dst_t = idx_pool.tile([P, tc_len], mybir.dt.int32, tag="dst")
                nc.sync.dma_start(dst_t[:, :cols], dsts_v[ch][:, :cols])

                # the HW contract for indirect DMA is one offset per
                # partition per instruction ([128,1] offsets -> [128,F] rows)
                msg_t = msg_pool.tile([P, tc_len * F], mybir.dt.float32, tag="msg")
                m3g = msg_t[:].rearrange("p (t f) -> p t f", f=F)
                for t in range(cols):
                    nc.gpsimd.indirect_dma_start(
                        out=m3g[:, t, :],
                        out_offset=None,
                        in_=x_aug.ap(),
                        in_offset=bass.IndirectOffsetOnAxis(
                            ap=src_t[:, t:t + 1], axis=0),
                    )

                keep_t = idx_pool.tile([P, tc_len], mybir.dt.float32, tag="keep")
                nc.vector.memset(keep_t[:, 0:1], 0.0)
                nc.vector.tensor_tensor(
                    out=keep_t[:, 1:cols],
                    in0=dst_t[:, 1:cols],
                    in1=dst_t[:, 0:cols - 1],
                    op=mybir.AluOpType.is_equal,
                )

                scan_t = scan_pool.tile([P, tc_len * F], mybir.dt.float32,
                                        tag="scan")
                m3 = msg_t[:].rearrange("p (t f) -> p t f", f=F)
                s3 = scan_t[:].rearrange("p (t f) -> p t f", f=F)
                for f in range(F):  # features + count col + pad col (zeros)
                    nc.vector.tensor_tensor_scan(
                        out=s3[:, :cols, f],
                        data0=keep_t[:, :cols],
                        data1=m3[:, :cols, f],
                        initial=0.0,
                        op0=mybir.AluOpType.mult,
                        op1=mybir.AluOpType.add,
                    )
                nc.sync.dma_start(pall_v[ch][:, :cols * F],
                                  scan_t[:, :cols * F])
                if cols < tc_len:
                    # define the unused tail columns of p_all (never read by
                    # close/carry gathers, but keeps the tensor fully
                    # initialized)
                    zf_t = epi_pool.tile([P, (tc_len - cols) * F],
                                         mybir.dt.float32, tag="zf")
                    nc.vector.memset(zf_t[:], 0.0)
                    nc.sync.dma_start(pall_v[ch][:, cols * F:], zf_t[:])

            # ---------------- epilogue ----------------
            cidx_t = epi_pool.tile([P, nj], mybir.dt.int32)
            nc.sync.dma_start(cidx_t[:],
                              close_idx.ap().rearrange("(p j) -> p j", j=nj))
            c2idx_t = epi_pool.tile([P, nj], mybir.dt.int32)
            nc.sync.dma_start(c2idx_t[:],
                              carry_idx.ap().rearrange("(p j) -> p j", j=nj))
            b_t = epi_pool.tile([P, nj * F], mybir.dt.float32)
            b3g = b_t[:].rearrange("p (j f) -> p j f", f=F)
            b2_t = epi_pool.tile([P, nj * F], mybir.dt.float32)
            b23g = b2_t[:].rearrange("p (j f) -> p j f", f=F)
            for j in range(nj):
                nc.gpsimd.indirect_dma_start(
                    out=b3g[:, j, :],
                    out_offset=None,
                    in_=p_all.ap(),
                    in_offset=bass.IndirectOffsetOnAxis(
                        ap=cidx_t[:, j:j + 1], axis=0),
                )
                # carry piece for nodes split across a row boundary (points
                # at the first piece's row-end slot; zero slot otherwise)
                nc.gpsimd.indirect_dma_start(
                    out=b23g[:, j, :],
                    out_offset=None,
                    in_=p_all.ap(),
                    in_offset=bass.IndirectOffsetOnAxis(
                        ap=c2idx_t[:, j:j + 1], axis=0),
                )
            nc.vector.tensor_add(out=b_t[:], in0=b_t[:], in1=b2_t[:])
            xsh_t = epi_pool.tile([P, nj * F], mybir.dt.float32)
            nc.sync.dma_start(
                xsh_t[:], xsh.ap().rearrange("(p j) f -> p (j f)", j=nj))
            w_t = epi_pool.tile([P, 2 * IN_DIM * HIDDEN + HIDDEN],
                                mybir.dt.float32)
            nc.sync.dma_start(w_t[:], wrep.ap())

            b3 = b_t[:].rearrange("p (j f) -> p j f", f=F)
            x3 = xsh_t[:].rearrange("p (j f) -> p j f", f=F)

            # rcp = 1 / max(count, 1)
            cnt_t = epi_pool.tile([P, nj], mybir.dt.float32)
            nc.vector.tensor_scalar_max(cnt_t[:], b3[:, :, IN_DIM], 1.0)
            rcp_t = epi_pool.tile([P, nj], mybir.dt.float32)
            nc.vector.reciprocal(rcp_t[:], cnt_t[:])

            out_t = epi_pool.tile([P, nj * HIDDEN], mybir.dt.float32)
            o3 = out_t[:].rearrange("p (j h) -> p j h", h=HIDDEN)
            acc_t = epi_pool.tile([P, nj], mybir.dt.float32)
            for h in range(HIDDEN):
                # acc = sum_f B_f * W_l[f,h]
                nc.vector.tensor_scalar_mul(
                    acc_t[:], b3[:, :, 0], w_t[:, 0 * HIDDEN + h:0 * HIDDEN + h + 1])
                for f in range(1, IN_DIM):
                    nc.vector.scalar_tensor_tensor(
                        out=acc_t[:],
                        in0=b3[:, :, f],
                        scalar=w_t[:, f * HIDDEN + h:f * HIDDEN + h + 1],
                        in1=acc_t[:],
                        op0=mybir.AluOpType.mult,
                        op1=mybir.AluOpType.add,
                    )
                # acc *= rcp  (mean); then add x @ W_r columns
                nc.vector.tensor_tensor(
                    out=acc_t[:], in0=acc_t[:], in1=rcp_t[:],
                    op=mybir.AluOpType.mult)
                wr0 = IN_DIM * HIDDEN
                for f in range(IN_DIM):
                    nc.vector.scalar_tensor_tensor(
                        out=acc_t[:],
                        in0=x3[:, :, f],
                        scalar=w_t[:, wr0 + f * HIDDEN + h:wr0 + f * HIDDEN + h + 1],
                        in1=acc_t[:],
                        op0=mybir.AluOpType.mult,
                        op1=mybir.AluOpType.add,
                    )
                # + b_l[h]
                bl0 = 2 * IN_DIM * HIDDEN
                nc.vector.tensor_scalar_add(
                    o3[:, :, h], acc_t[:], w_t[:, bl0 + h:bl0 + h + 1])

            nc.sync.dma_start(
                out_d.ap().rearrange("(p j) h -> p (j h)", j=nj), out_t[:])

    nc.compile()
    return nc


# ---------------------------------------------------------------- host
def pack_core(src_c, dst_local, n_nodes_pc, npad, tc_len, e_pack):
    """Pack one core's dst-sorted edges into fixed-width partition rows.

    src_c:     [Ec] global src node id per edge, sorted by dst_local
    dst_local: [Ec] local (0-based) dst node id per edge, sorted ascending
    Returns (srcs_packed, dsts_packed, close_idx) as int32 arrays.
    """
    ec = src_c.shape[0]
    tw = tc_len // 4                 # used columns per row in the tail chunk
    ts = e_pack - 128 * tc_len       # first slot of the tail chunk
    degs = np.bincount(dst_local, minlength=n_nodes_pc)
    assert degs.max() < tw, f"in-degree {degs.max()} exceeds tail row width"
    assert ec < ts + 128 * tw, f"edges {ec} exceed packed capacity"

    def slot_of(e):
        e = np.asarray(e, np.int64)
        return np.where(e < ts, e, ts + ((e - ts) // tw) * tc_len + (e - ts) % tw)

    # dense layout: edge rank e -> slot_of(e); rows fill completely, a node
    # may straddle one row boundary (deg < row width), giving two scan
    # pieces combined via the carry gather.
    srcs_packed = np.full(e_pack, PAD_ROW, np.int32)
    dsts_packed = np.full(e_pack, -1, np.int32)
    pos = slot_of(np.arange(ec))
    srcs_packed[pos] = src_c.astype(np.int32)
    dsts_packed[pos] = dst_local.astype(np.int32)

    seg_start = np.zeros(n_nodes_pc + 1, np.int64)
    np.cumsum(degs, out=seg_start[1:])
    start = seg_start[:-1]
    end = seg_start[1:] - 1          # inclusive last edge (start-1 if deg 0)
    has_edge = degs > 0
    s0 = slot_of(np.maximum(start, 0))
    s1 = slot_of(np.maximum(end, 0))
    row0 = s0 // tc_len
    split = has_edge & (row0 != s1 // tc_len)
    zero_slot = int(slot_of(ec))     # first pad slot: keep=0, msg=0 -> scan 0

    close_idx = np.full(npad, zero_slot, np.int32)
    close_idx[:n_nodes_pc][has_edge] = s1[has_edge].astype(np.int32)
    carry_idx = np.full(npad, zero_slot, np.int32)
    carry_idx[:n_nodes_pc][split] = (
        row0[split] * tc_len
        + np.where(s0[split] >= ts, tw, tc_len) - 1).astype(np.int32)
    return srcs_packed, dsts_packed, close_idx, carry_idx


def prepare_inputs(x, edge_index, W_l, b_l, W_r):
    x = np.asarray(x, np.float32)
    W_l = np.asarray(W_l, np.float32)
    b_l = np.asarray(b_l, np.float32)
    W_r = np.asarray(W_r, np.float32)
    src = np.asarray(edge_index[0])
    dst = np.asarray(edge_index[1])

    x_aug = np.zeros((N_NODES + 1, F), np.float32)
    x_aug[:N_NODES, :IN_DIM] = x
    x_aug[:N_NODES, IN_DIM] = 1.0

    wcat = np.concatenate([W_l.reshape(-1), W_r.reshape(-1), b_l.reshape(-1)])
    wrep = np.ascontiguousarray(np.broadcast_to(wcat, (128, wcat.shape[0])),
                                np.float32)

    order = np.argsort(dst, kind="stable")
    dst_s = dst[order]
    src_s = src[order]
    bounds = np.searchsorted(dst_s, np.arange(0, N_NODES + 1, NODES_PC))

    in_maps = []
    for c in range(N_CORES):
        lo, hi = bounds[c], bounds[c + 1]
        src_c = src_s[lo:hi]
        dst_l = (dst_s[lo:hi] - c * NODES_PC).astype(np.int64)
        sp, dp, ci, c2i = pack_core(src_c, dst_l, NODES_PC, NPAD, TC, E_PACK)
        xsh = np.zeros((NPAD, F), np.float32)
        xsh[:NODES_PC] = x_aug[c * NODES_PC:(c + 1) * NODES_PC]
        in_maps.append({
            "x_aug": x_aug, "srcs": sp, "dsts": dp, "close_idx": ci,
            "carry_idx": c2i, "xsh": xsh, "wrep": wrep,
        })
    return in_maps


# ---------------------------------------------------------------- runner
class SpmdRunner:
    def __init__(self, nc, n_cores):
        import jax
        from jax.sharding import Mesh, PartitionSpec
        from jax.experimental.shard_map import shard_map
        from concourse.bass2jax import (
            _bass_exec_p, install_neuronx_cc_hook, partition_id_tensor)

        install_neuronx_cc_hook()
        self.n_cores = n_cores
        pname = nc.partition_id_tensor.name if nc.partition_id_tensor else None
        in_names, out_names, out_avals, zero_outs = [], [], [], []
        for alloc in nc.m.functions[0].allocations:
            if not isinstance(alloc, mybir.MemoryLocationSet):
                continue
            name = alloc.memorylocations[0].name
            if alloc.kind == "ExternalInput":
                if name != pname:
                    in_names.append(name)
            elif alloc.kind == "ExternalOutput":
                out_names.append(name)
                shape = tuple(alloc.tensor_shape)
                dt_np = mybir.dt.np(alloc.dtype)
                out_avals.append(jax.core.ShapedArray(shape, dt_np))
                zero_outs.append(np.zeros(shape, dt_np))
        self.in_names, self.out_names = in_names, out_names
        self.zero_outs = zero_outs
        n_params, n_outs = len(in_names), len(out_names)
        all_names = in_names + out_names + ([pname] if pname else [])

        def _body(*args):
            operands = list(args)
            if pname is not None:
                operands.append(partition_id_tensor())
            return tuple(_bass_exec_p.bind(
                *operands, out_avals=tuple(out_avals),
                in_names=tuple(all_names), out_names=tuple(out_names),
                lowering_input_output_aliases=(),
                sim_require_finite=True, sim_require_nnan=True, nc=nc))

        devices = jax.devices()[:n_cores]
        mesh = Mesh(np.asarray(devices), ("core",))
        self._mesh = mesh
        specs_in = (PartitionSpec("core"),) * (n_params + n_outs)
        specs_out = (PartitionSpec("core"),) * n_outs
        self._fn = jax.jit(
            shard_map(_body, mesh=mesh, in_specs=specs_in,
                      out_specs=specs_out, check_rep=False),
            keep_unused=True)
        self._jax = jax

    def prepare(self, in_maps):
        per = [[np.asarray(m[n]) for n in self.in_names] for m in in_maps]
        cat = [np.concatenate([per[c][i] for c in range(self.n_cores)], axis=0)
               for i in range(len(self.in_names))]
        cat += [np.concatenate([z] * self.n_cores, axis=0)
                for z in self.zero_outs]
        return cat

    def device_put(self, args):
        """Ship prepared args to the devices once (for repeat timing)."""
        import jax
        from jax.sharding import NamedSharding, PartitionSpec
        mesh = self._mesh
        sh = NamedSharding(mesh, PartitionSpec("core"))
        out = [jax.device_put(a, sh) for a in args]
        jax.block_until_ready(out)
        return out

    def run(self, args):
        outs = self._fn(*args)
        self._jax.block_until_ready(outs)
        return outs

    def results(self, outs):
        res = [dict() for _ in range(self.n_cores)]
        for i, name in enumerate(self.out_names):
            for c, part in enumerate(
                    np.split(np.asarray(outs[i]), self.n_cores, axis=0)):
                res[c][name] = part
        return res


_CACHE = {}


def kernel(x, edge_index, W_l, b_l, W_r):
    if "runner" not in _CACHE:
        nc = build_program()
        _CACHE["runner"] = SpmdRunner(nc, N_CORES)
    runner = _CACHE["runner"]
    in_maps = prepare_inputs(x, edge_index, W_l, b_l, W_r)
    args = runner.prepare(in_maps)
    res = runner.results(runner.run(args))
    out = np.empty((N_NODES, HIDDEN), np.float32)
    for c in range(N_CORES):
        out[c * NODES_PC:(c + 1) * NODES_PC] = res[c]["out"][:NODES_PC]
    return out



# revision 13
# speedup vs baseline: 3.6715x; 3.6715x over previous
"""SAGEConv-style GNN message passing on 8 Trainium2 NeuronCores.

out = (mean_{j in N(i)} x_j) @ W_l + b_l + x_i @ W_r
with N(i) defined by edge_index ([2, E]: src=row0, dst=row1), mean over
in-edges (segment mean by dst), N=100000 nodes, E=6400000 edges.

Distribution: shard by DESTINATION node range - core c owns nodes
[c*12500, (c+1)*12500) and receives exactly the edges targeting its range.
Each core computes its output slice independently; no collective needed.

Device algorithm per core (the gather is the whole game):
  1. The per-edge feature gather runs through the GPSIMD bulk gather
     (InstDMAGatherAnt): one instruction fetches 8192 rows of a bf16
     feature table (256B rows) via an int16 index stream.  int16 indices
     limit a table to 32k rows, so the 100k-node table is split into 4
     banks and each core's edges are grouped by src bank.  Hardware
     constraints found empirically: <=1024 idxs/instruction with
     single_packet=True, <=8192 with single_packet=False (SWDGE ring is
     firmware-sized; bigger instructions hang the device).
  2. Edges are binned by dst node, each node's edges occupying W=deg
     consecutive column slots of one partition row (64 columns per gather
     tile).  A precomputed keep-mask drives a segmented scan
     (state = keep*state + msg, fp32 state) over each tile; the scan value
     at a node's last slot is its [sum(x_j), count] - positions precomputed
     on host.
  3. Scan tiles are stored to a DRAM scratch (p_all, bf16); per-node close
     rows are fetched back with one-offset-per-partition indirect DMAs in
     node-major order, summed over the 4 banks.
  4. Epilogue: out = (S/max(cnt,1)) @ W_l + b_l + x_i @ W_r via per-column
     scalar_tensor_tensor ops, as in the row-gather baseline.

Host work is index/layout only: bucketing edges, bin-packing nodes into
partition rows, building idx/keep/close streams, dtype conversion of the
replicated feature table.  All value arithmetic runs on device.
"""

import numpy as np
import ml_dtypes

import concourse.bass as bass
import concourse.tile as tile
from concourse import bacc, mybir

BF16 = ml_dtypes.bfloat16

# ---------------------------------------------------------------- config
N_NODES = 100000
N_EDGES = 6400000
IN_DIM = 10
HIDDEN = 16
N_CORES = 8

NODES_PC = N_NODES // N_CORES   # 12500
NPAD = 12544                    # = 128 * 98 padded nodes per core
NJ = NPAD // 128                # 98

NBANKS = 4
BANK_N = 25000                  # real nodes per bank
BANK_STRIDE = 25004             # bank rows incl. zero rows
ZROW = 25000                    # zero row (local) in each bank
XT_ROWS = NBANKS * BANK_STRIDE  # 102416
ES = 128                        # bf16 elems per table row (256B)

IDX_PI = 8192                   # indices per gather instruction
COLS = IDX_PI // 128            # 64 columns per gather tile
TILES_PB = 25                   # gather instructions per (core, bank)
RCOLS = TILES_PB * COLS         # 1600 columns per partition row per bank
SLOTS_PB = IDX_PI * TILES_PB    # 204800 slots per (core, bank)
PALL_W = 16                     # p_all row width (bf16)
PALL_ROWS = NBANKS * TILES_PB * 128 * COLS  # 819200
WREP_W = 2 * IN_DIM * HIDDEN + HIDDEN


# ---------------------------------------------------------------- device
def build_program(num_devices=N_CORES):
    """Build the per-core Bass program (SPMD: same program, per-core data)."""
    P = 128
    nc = bacc.Bacc("TRN2", target_bir_lowering=False, debug=False,
                   num_devices=num_devices)

    x_t = nc.dram_tensor("x_t", [XT_ROWS, ES], mybir.dt.bfloat16,
                         kind="ExternalInput")
    idxs = nc.dram_tensor("idxs", [NBANKS, TILES_PB, P, IDX_PI // 16],
                          mybir.dt.int16, kind="ExternalInput")
    keeps = nc.dram_tensor("keeps", [NBANKS, TILES_PB, P, COLS],
                           mybir.dt.bfloat16, kind="ExternalInput")
    cidx = nc.dram_tensor("cidx", [P, NBANKS * NJ], mybir.dt.int32,
                          kind="ExternalInput")
    xsh = nc.dram_tensor("xsh", [NPAD, 12], mybir.dt.float32,
                         kind="ExternalInput")
    wrep = nc.dram_tensor("wrep", [P, WREP_W], mybir.dt.float32,
                          kind="ExternalInput")
    p_all = nc.dram_tensor("p_all", [PALL_ROWS, PALL_W], mybir.dt.bfloat16,
                           kind="Internal")
    out_d = nc.dram_tensor("out", [NPAD, HIDDEN], mybir.dt.float32,
                           kind="ExternalOutput")

    pall_v = p_all.ap().rearrange("(b t p c) k -> b t p (c k)",
                                  b=NBANKS, t=TILES_PB, p=P)

    with tile.TileContext(nc) as tc:
        with (
            tc.tile_pool(name="g", bufs=3) as g_pool,
            tc.tile_pool(name="misc", bufs=3) as misc_pool,
            tc.tile_pool(name="cl", bufs=2) as cl_pool,
            tc.tile_pool(name="epi", bufs=1) as epi_pool,
        ):
            cidx_t = epi_pool.tile([P, NBANKS * NJ], mybir.dt.int32)
            nc.sync.dma_start(cidx_t[:], cidx.ap())
            s_t = epi_pool.tile([P, NJ * PALL_W], mybir.dt.float32)
            nc.vector.memset(s_t[:], 0.0)

            for b in range(NBANKS):
                src = x_t.ap()[b * BANK_STRIDE:(b + 1) * BANK_STRIDE]
                prev_s3 = None
                for t in range(TILES_PB):
                    idx_t = misc_pool.tile([P, IDX_PI // 16], mybir.dt.int16,
                                           tag="idx")
                    nc.sync.dma_start(idx_t[:], idxs.ap()[b, t])
                    keep_t = misc_pool.tile([P, COLS], mybir.dt.bfloat16,
                                            tag="keep")
                    nc.sync.dma_start(keep_t[:], keeps.ap()[b, t])

                    g_t = g_pool.tile([P, COLS * ES], mybir.dt.bfloat16,
                                      tag="g")
                    g3 = g_t[:].rearrange("p (c e) -> p c e", e=ES)
                    nc.gpsimd.dma_gather(g3, src, idx_t[:], IDX_PI, IDX_PI,
                                         ES, single_packet=False)

                    scan_t = g_pool.tile([P, COLS * PALL_W],
                                         mybir.dt.bfloat16, tag="scan")
                    s3 = scan_t[:].rearrange("p (c k) -> p c k", k=PALL_W)
                    # lanes 11..15 of the table are zero, so scanning all 16
                    # lanes leaves p_all fully initialized.  The scan chains
                    # across tiles: initial = previous tile's last column.
                    for f in range(PALL_W):
                        nc.vector.tensor_tensor_scan(
                            out=s3[:, :, f],
                            data0=keep_t[:],
                            data1=g3[:, :, f],
                            initial=(0.0 if prev_s3 is None
                                     else prev_s3[:, COLS - 1:COLS, f]),
                            op0=mybir.AluOpType.mult,
                            op1=mybir.AluOpType.add,
                        )
                    nc.sync.dma_start(pall_v[b, t], scan_t[:])
                    prev_s3 = s3

                # close rows for this bank (node-major [p, j])
                cl_t = cl_pool.tile([P, NJ * PALL_W], mybir.dt.bfloat16,
                                    tag="cl")
                cl3 = cl_t[:].rearrange("p (j k) -> p j k", k=PALL_W)
                for j in range(NJ):
                    nc.gpsimd.indirect_dma_start(
                        out=cl3[:, j, :],
                        out_offset=None,
                        in_=p_all.ap(),
                        in_offset=bass.IndirectOffsetOnAxis(
                            ap=cidx_t[:, b * NJ + j:b * NJ + j + 1], axis=0),
                    )
                clf_t = cl_pool.tile([P, NJ * PALL_W], mybir.dt.float32,
                                     tag="clf")
                nc.vector.tensor_scalar_mul(clf_t[:], cl_t[:], 1.0)
                nc.vector.tensor_tensor(out=s_t[:], in0=s_t[:], in1=clf_t[:],
                                        op=mybir.AluOpType.add)

            # ---------------- epilogue ----------------
            xsh_t = epi_pool.tile([P, NJ * 12], mybir.dt.float32)
            nc.sync.dma_start(
                xsh_t[:], xsh.ap().rearrange("(p j) f -> p (j f)", j=NJ))
            w_t = epi_pool.tile([P, WREP_W], mybir.dt.float32)
            nc.sync.dma_start(w_t[:], wrep.ap())

            s3e = s_t[:].rearrange("p (j k) -> p j k", k=PALL_W)
            x3 = xsh_t[:].rearrange("p (j f) -> p j f", f=12)

            cnt_t = epi_pool.tile([P, NJ], mybir.dt.float32)
            nc.vector.tensor_scalar_max(cnt_t[:], s3e[:, :, IN_DIM], 1.0)
            rcp_t = epi_pool.tile([P, NJ], mybir.dt.float32)
            nc.vector.reciprocal(rcp_t[:], cnt_t[:])

            out_t = epi_pool.tile([P, NJ * HIDDEN], mybir.dt.float32)
            o3 = out_t[:].rearrange("p (j h) -> p j h", h=HIDDEN)
            acc_t = epi_pool.tile([P, NJ], mybir.dt.float32)
            for h in range(HIDDEN):
                nc.vector.tensor_scalar_mul(
                    acc_t[:], s3e[:, :, 0], w_t[:, h:h + 1])
                for f in range(1, IN_DIM):
                    nc.vector.scalar_tensor_tensor(
                        out=acc_t[:],
                        in0=s3e[:, :, f],
                        scalar=w_t[:, f * HIDDEN + h:f * HIDDEN + h + 1],
                        in1=acc_t[:],
                        op0=mybir.AluOpType.mult,
                        op1=mybir.AluOpType.add,
                    )
                nc.vector.tensor_tensor(
                    out=acc_t[:], in0=acc_t[:], in1=rcp_t[:],
                    op=mybir.AluOpType.mult)
                wr0 = IN_DIM * HIDDEN
                for f in range(IN_DIM):
                    nc.vector.scalar_tensor_tensor(
                        out=acc_t[:],
                        in0=x3[:, :, f],
                        scalar=w_t[:, wr0 + f * HIDDEN + h:wr0 + f * HIDDEN + h + 1],
                        in1=acc_t[:],
                        op0=mybir.AluOpType.mult,
                        op1=mybir.AluOpType.add,
                    )
                bl0 = 2 * IN_DIM * HIDDEN
                nc.vector.tensor_scalar_add(
                    o3[:, :, h], acc_t[:], w_t[:, bl0 + h:bl0 + h + 1])

            nc.sync.dma_start(
                out_d.ap().rearrange("(p j) h -> p (j h)", j=NJ), out_t[:])

    nc.compile()
    return nc


# ---------------------------------------------------------------- host
def _pack_bank(deg):
    """Assign nodes to 128 partition rows of RCOLS columns (scan chains
    across the bank's gather tiles, so each partition is one long row).

    Widths are dealt snake-wise in descending order so row loads balance
    to within a few columns; within a row, columns pack densely in deal
    order.  Returns (part, col0) per node.
    """
    w = np.maximum(deg, 1).astype(np.int64)
    order = np.argsort(-w, kind="stable")
    w_desc = w[order]
    loads = np.zeros(128, np.int64)
    part_sorted = np.empty(NPAD, np.int64)
    for r in range(NJ):          # round-based LPT: widest -> lightest row
        rows = np.argsort(loads, kind="stable")
        part_sorted[r * 128:(r + 1) * 128] = rows
        loads[rows] += w_desc[r * 128:(r + 1) * 128]
    part = np.empty(NPAD, np.int64)
    part[order] = part_sorted

    # segmented exclusive cumsum of widths per row, in deal order
    ord2 = np.lexsort((np.arange(NPAD), part_sorted))
    nodes2 = order[ord2]
    ws = w[nodes2]
    csum = np.cumsum(ws)
    row_of2 = part_sorted[ord2]
    starts = np.searchsorted(row_of2, np.arange(128))
    base = np.zeros(128, np.int64)
    base[1:] = csum[starts[1:] - 1]
    col0_2 = csum - ws - base[row_of2]
    col0 = np.empty(NPAD, np.int64)
    col0[nodes2] = col0_2
    load = csum[np.append(starts[1:] - 1, NPAD - 1)] - base
    assert load.max() <= RCOLS, f"row overflow: {load.max()} > {RCOLS}"
    return part, col0


def prepare_inputs(x, edge_index, W_l, b_l, W_r):
    x = np.asarray(x, np.float32)
    W_l = np.asarray(W_l, np.float32)
    b_l = np.asarray(b_l, np.float32)
    W_r = np.asarray(W_r, np.float32)
    src = np.asarray(edge_index[0], np.int64)
    dst = np.asarray(edge_index[1], np.int64)

    # bf16 feature table: 4 banks of 25604 rows (row 25600+ of each bank = 0)
    x_tab = np.zeros((XT_ROWS, ES), np.float32)
    for b in range(NBANKS):
        lo = b * BANK_N
        hi = min((b + 1) * BANK_N, N_NODES)
        if hi > lo:
            rows = b * BANK_STRIDE + np.arange(hi - lo)
            x_tab[rows, :IN_DIM] = x[lo:hi]
            x_tab[rows, IN_DIM] = 1.0
    x_tab = x_tab.astype(BF16)

    wcat = np.concatenate([W_l.reshape(-1), W_r.reshape(-1), b_l.reshape(-1)])
    wrep = np.ascontiguousarray(
        np.broadcast_to(wcat, (128, wcat.shape[0])), np.float32)

    core_of = dst // NODES_PC
    bank_of = src // BANK_N
    in_maps = []
    for c in range(N_CORES):
        sel_c = core_of == c
        src_c = src[sel_c]
        dst_c = dst[sel_c] - c * NODES_PC
        bank_c = bank_of[sel_c]

        idx_all = np.empty((NBANKS, TILES_PB, 128, IDX_PI // 16), np.int16)
        keep_all = np.zeros((NBANKS, SLOTS_PB), np.float32)
        cidx_all = np.empty((NBANKS, 128, NJ), np.int32)

        for b in range(NBANKS):
            sel_b = bank_c == b
            sb = (src_c[sel_b] - b * BANK_N).astype(np.int64)
            db = dst_c[sel_b]
            order = np.argsort(db, kind="stable")
            sb = sb[order]
            db = db[order]
            deg = np.bincount(db, minlength=NPAD)
            w = np.maximum(deg, 1)
            assert w.sum() <= SLOTS_PB, f"slots {w.sum()} exceed {SLOTS_PB}"
            part, col0 = _pack_bank(deg)

            # per-edge slot: global col -> (tile, col%COLS, partition)
            seg = np.zeros(NPAD + 1, np.int64)
            np.cumsum(deg, out=seg[1:])
            rank = np.arange(db.shape[0]) - seg[db]
            col = col0[db] + rank
            p = part[db]
            pos = (col // COLS) * IDX_PI + (col % COLS) * 128 + p

            stream = np.full(SLOTS_PB, ZROW, np.int64)
            stream[pos] = sb
            keep_all[b][pos[rank > 0]] = 1.0

            # close p_all row: ((b*T + tile)*128 + p)*COLS + col%COLS
            cl_col = col0 + w - 1
            cl_row = (((b * TILES_PB + cl_col // COLS) * 128 + part) * COLS
                      + cl_col % COLS)
            cidx_all[b] = cl_row.reshape(128, NJ).astype(np.int32)

            # wrap: instruction t gets stream[t*IDX_PI:(t+1)*IDX_PI];
            # position i -> partition i%16 (replicated x8), column i//16
            st = stream.reshape(TILES_PB, IDX_PI // 16, 16).astype(np.int16)
            wt = np.transpose(st, (0, 2, 1))          # [T, 16, IDX_PI//16]
            idx_all[b] = np.tile(wt, (1, 8, 1))

        keeps = keep_all.reshape(NBANKS, TILES_PB, COLS, 128).transpose(
            0, 1, 3, 2)  # [b, t, p, c]
        xsh = np.zeros((NPAD, 12), np.float32)
        xsh[:NODES_PC, :IN_DIM] = x[c * NODES_PC:(c + 1) * NODES_PC]
        in_maps.append({
            "x_t": x_tab,
            "idxs": idx_all,
            "keeps": np.ascontiguousarray(keeps).astype(BF16),
            "cidx": np.ascontiguousarray(
                cidx_all.transpose(1, 0, 2).reshape(128, NBANKS * NJ)),
            "xsh": xsh,
            "wrep": wrep,
        })
    return in_maps


# ---------------------------------------------------------------- runner
class SpmdRunner:
    def __init__(self, nc, n_cores):
        import jax
        from jax.sharding import Mesh, PartitionSpec
        from jax.experimental.shard_map import shard_map
        from concourse.bass2jax import (
            _bass_exec_p, install_neuronx_cc_hook, partition_id_tensor)

        install_neuronx_cc_hook()
        self.n_cores = n_cores
        pname = nc.partition_id_tensor.name if nc.partition_id_tensor else None
        in_names, out_names, out_avals, zero_outs = [], [], [], []
        for alloc in nc.m.functions[0].allocations:
            if not isinstance(alloc, mybir.MemoryLocationSet):
                continue
            name = alloc.memorylocations[0].name
            if alloc.kind == "ExternalInput":
                if name != pname:
                    in_names.append(name)
            elif alloc.kind == "ExternalOutput":
                out_names.append(name)
                shape = tuple(alloc.tensor_shape)
                dt_np = mybir.dt.np(alloc.dtype)
                out_avals.append(jax.core.ShapedArray(shape, dt_np))
                zero_outs.append(np.zeros(shape, dt_np))
        self.in_names, self.out_names = in_names, out_names
        self.zero_outs = zero_outs
        n_params, n_outs = len(in_names), len(out_names)
        all_names = in_names + out_names + ([pname] if pname else [])

        def _body(*args):
            operands = list(args)
            if pname is not None:
                operands.append(partition_id_tensor())
            return tuple(_bass_exec_p.bind(
                *operands, out_avals=tuple(out_avals),
                in_names=tuple(all_names), out_names=tuple(out_names),
                lowering_input_output_aliases=(),
                sim_require_finite=True, sim_require_nnan=True, nc=nc))

        devices = jax.devices()[:n_cores]
        mesh = Mesh(np.asarray(devices), ("core",))
        self._mesh = mesh
        specs_in = (PartitionSpec("core"),) * (n_params + n_outs)
        specs_out = (PartitionSpec("core"),) * n_outs
        self._fn = jax.jit(
            shard_map(_body, mesh=mesh, in_specs=specs_in,
                      out_specs=specs_out, check_rep=False),
            keep_unused=True)
        self._jax = jax

    def prepare(self, in_maps):
        per = [[np.asarray(m[n]) for n in self.in_names] for m in in_maps]
        cat = [np.concatenate([per[c][i] for c in range(self.n_cores)], axis=0)
               for i in range(len(self.in_names))]
        cat += [np.concatenate([z] * self.n_cores, axis=0)
                for z in self.zero_outs]
        return cat

    def device_put(self, args):
        """Ship prepared args to the devices once (for repeat timing)."""
        import jax
        from jax.sharding import NamedSharding, PartitionSpec
        mesh = self._mesh
        sh = NamedSharding(mesh, PartitionSpec("core"))
        out = [jax.device_put(a, sh) for a in args]
        jax.block_until_ready(out)
        return out

    def run(self, args):
        outs = self._fn(*args)
        self._jax.block_until_ready(outs)
        return outs

    def results(self, outs):
        res = [dict() for _ in range(self.n_cores)]
        for i, name in enumerate(self.out_names):
            for c, part in enumerate(
                    np.split(np.asarray(outs[i]), self.n_cores, axis=0)):
                res[c][name] = part
        return res


_CACHE = {}


def kernel(x, edge_index, W_l, b_l, W_r):
    if "runner" not in _CACHE:
        nc = build_program()
        _CACHE["runner"] = SpmdRunner(nc, N_CORES)
    runner = _CACHE["runner"]
    in_maps = prepare_inputs(x, edge_index, W_l, b_l, W_r)
    args = runner.prepare(in_maps)
    res = runner.results(runner.run(args))
    out = np.empty((N_NODES, HIDDEN), np.float32)
    for c in range(N_CORES):
        out[c * NODES_PC:(c + 1) * NODES_PC] = res[c]["out"][:NODES_PC]
    return out


# revision 17
# speedup vs baseline: 3.6745x; 1.0008x over previous
"""SAGEConv-style GNN message passing on 8 Trainium2 NeuronCores.

out = (mean_{j in N(i)} x_j) @ W_l + b_l + x_i @ W_r
with N(i) defined by edge_index ([2, E]: src=row0, dst=row1), mean over
in-edges (segment mean by dst), N=100000 nodes, E=6400000 edges.

Distribution: shard by DESTINATION node range - core c owns nodes
[c*12500, (c+1)*12500) and receives exactly the edges targeting its range.
Each core computes its output slice independently; no collective needed.

Device algorithm per core (the gather is the whole game):
  1. The per-edge feature gather runs through the GPSIMD bulk gather
     (InstDMAGatherAnt): one instruction fetches 8192 rows of a bf16
     feature table (256B rows) via an int16 index stream.  int16 indices
     limit a table to 32k rows, so the 100k-node table is split into 4
     banks and each core's edges are grouped by src bank.  Hardware
     constraints found empirically: <=1024 idxs/instruction with
     single_packet=True, <=8192 with single_packet=False (SWDGE ring is
     firmware-sized; bigger instructions hang the device).
  2. Edges are binned by dst node, each node's edges occupying W=deg
     consecutive column slots of one partition row (64 columns per gather
     tile).  A precomputed keep-mask drives a segmented scan
     (state = keep*state + msg, fp32 state) over each tile; the scan value
     at a node's last slot is its [sum(x_j), count] - positions precomputed
     on host.
  3. Scan tiles are stored to a DRAM scratch (p_all, bf16); per-node close
     rows are fetched back with one-offset-per-partition indirect DMAs in
     node-major order, summed over the 4 banks.
  4. Epilogue: out = (S/max(cnt,1)) @ W_l + b_l + x_i @ W_r via per-column
     scalar_tensor_tensor ops, as in the row-gather baseline.

Host work is index/layout only: bucketing edges, bin-packing nodes into
partition rows, building idx/keep/close streams, dtype conversion of the
replicated feature table.  All value arithmetic runs on device.
"""

import numpy as np
import ml_dtypes

import concourse.bass as bass
import concourse.tile as tile
from concourse import bacc, mybir

BF16 = ml_dtypes.bfloat16

# ---------------------------------------------------------------- config
N_NODES = 100000
N_EDGES = 6400000
IN_DIM = 10
HIDDEN = 16
N_CORES = 8

NODES_PC = N_NODES // N_CORES   # 12500
NPAD = 12544                    # = 128 * 98 padded nodes per core
NJ = NPAD // 128                # 98

NBANKS = 4
BANK_N = 25000                  # real nodes per bank
BANK_STRIDE = 25004             # bank rows incl. zero rows
ZROW = 25000                    # zero row (local) in each bank
XT_ROWS = NBANKS * BANK_STRIDE  # 102416
ES = 128                        # bf16 elems per table row (256B)

IDX_PI = 8192                   # indices per gather instruction
COLS = IDX_PI // 128            # 64 columns per gather tile
TILES_PB = 25                   # gather instructions per (core, bank)
RCOLS = TILES_PB * COLS         # 1600 columns per partition row per bank
SLOTS_PB = IDX_PI * TILES_PB    # 204800 slots per (core, bank)
PALL_W = 12                     # p_all row width (bf16)
PALL_ROWS = NBANKS * TILES_PB * 128 * COLS  # 819200
WREP_W = 2 * IN_DIM * HIDDEN + HIDDEN


# ---------------------------------------------------------------- device
def build_program(num_devices=N_CORES):
    """Build the per-core Bass program (SPMD: same program, per-core data)."""
    P = 128
    nc = bacc.Bacc("TRN2", target_bir_lowering=False, debug=False,
                   num_devices=num_devices)

    x_t = nc.dram_tensor("x_t", [XT_ROWS, ES], mybir.dt.bfloat16,
                         kind="ExternalInput")
    idxs = nc.dram_tensor("idxs", [NBANKS, TILES_PB, P, IDX_PI // 16],
                          mybir.dt.int16, kind="ExternalInput")
    keeps = nc.dram_tensor("keeps", [NBANKS, TILES_PB, P, COLS],
                           mybir.dt.bfloat16, kind="ExternalInput")
    cidx = nc.dram_tensor("cidx", [P, NBANKS * NJ], mybir.dt.int32,
                          kind="ExternalInput")
    xsh = nc.dram_tensor("xsh", [NPAD, 12], mybir.dt.float32,
                         kind="ExternalInput")
    wrep = nc.dram_tensor("wrep", [P, WREP_W], mybir.dt.float32,
                          kind="ExternalInput")
    p_all = nc.dram_tensor("p_all", [PALL_ROWS, PALL_W], mybir.dt.bfloat16,
                           kind="Internal")
    out_d = nc.dram_tensor("out", [NPAD, HIDDEN], mybir.dt.float32,
                           kind="ExternalOutput")

    pall_v = p_all.ap().rearrange("(b t p c) k -> b t p (c k)",
                                  b=NBANKS, t=TILES_PB, p=P)

    with tile.TileContext(nc) as tc:
        with (
            tc.tile_pool(name="g", bufs=3) as g_pool,
            tc.tile_pool(name="misc", bufs=3) as misc_pool,
            tc.tile_pool(name="cl", bufs=2) as cl_pool,
            tc.tile_pool(name="epi", bufs=1) as epi_pool,
        ):
            cidx_t = epi_pool.tile([P, NBANKS * NJ], mybir.dt.int32)
            nc.sync.dma_start(cidx_t[:], cidx.ap())
            s_t = epi_pool.tile([P, NJ * PALL_W], mybir.dt.float32)
            nc.vector.memset(s_t[:], 0.0)

            for b in range(NBANKS):
                src = x_t.ap()[b * BANK_STRIDE:(b + 1) * BANK_STRIDE]
                prev_s3 = None
                cl_t = cl_pool.tile([P, NJ * PALL_W], mybir.dt.bfloat16,
                                    tag="cl")
                cl3 = cl_t[:].rearrange("p (j k) -> p j k", k=PALL_W)

                def emit_close(j, b=b, cl3=cl3):
                    nc.gpsimd.indirect_dma_start(
                        out=cl3[:, j, :],
                        out_offset=None,
                        in_=p_all.ap(),
                        in_offset=bass.IndirectOffsetOnAxis(
                            ap=cidx_t[:, b * NJ + j:b * NJ + j + 1], axis=0),
                    )

                for t in range(TILES_PB):
                    idx_t = misc_pool.tile([P, IDX_PI // 16], mybir.dt.int16,
                                           tag="idx")
                    nc.sync.dma_start(idx_t[:], idxs.ap()[b, t])
                    keep_t = misc_pool.tile([P, COLS], mybir.dt.bfloat16,
                                            tag="keep")
                    nc.sync.dma_start(keep_t[:], keeps.ap()[b, t])

                    g_t = g_pool.tile([P, COLS * ES], mybir.dt.bfloat16,
                                      tag="g")
                    g3 = g_t[:].rearrange("p (c e) -> p c e", e=ES)
                    nc.gpsimd.dma_gather(g3, src, idx_t[:], IDX_PI, IDX_PI,
                                         ES, single_packet=False)

                    scan_t = g_pool.tile([P, COLS * PALL_W],
                                         mybir.dt.bfloat16, tag="scan")
                    s3 = scan_t[:].rearrange("p (c k) -> p c k", k=PALL_W)
                    # lane 11 of the table is zero, so scanning 12 lanes
                    # leaves p_all fully initialized.  The scan chains
                    # across tiles: initial = previous tile's last column.
                    for f in range(PALL_W):
                        nc.vector.tensor_tensor_scan(
                            out=s3[:, :, f],
                            data0=keep_t[:],
                            data1=g3[:, :, f],
                            initial=(0.0 if prev_s3 is None
                                     else prev_s3[:, COLS - 1:COLS, f]),
                            op0=mybir.AluOpType.mult,
                            op1=mybir.AluOpType.add,
                        )
                    nc.sync.dma_start(pall_v[b, t], scan_t[:])
                    prev_s3 = s3

                for j in range(NJ):
                    emit_close(j)
                clf_t = cl_pool.tile([P, NJ * PALL_W], mybir.dt.float32,
                                     tag="clf")
                nc.vector.tensor_scalar_mul(clf_t[:], cl_t[:], 1.0)
                nc.vector.tensor_tensor(out=s_t[:], in0=s_t[:], in1=clf_t[:],
                                        op=mybir.AluOpType.add)

            # ---------------- epilogue ----------------
            xsh_t = epi_pool.tile([P, NJ * 12], mybir.dt.float32)
            nc.sync.dma_start(
                xsh_t[:], xsh.ap().rearrange("(p j) f -> p (j f)", j=NJ))
            w_t = epi_pool.tile([P, WREP_W], mybir.dt.float32)
            nc.sync.dma_start(w_t[:], wrep.ap())

            s3e = s_t[:].rearrange("p (j k) -> p j k", k=PALL_W)
            x3 = xsh_t[:].rearrange("p (j f) -> p j f", f=12)

            cnt_t = epi_pool.tile([P, NJ], mybir.dt.float32)
            nc.vector.tensor_scalar_max(cnt_t[:], s3e[:, :, IN_DIM], 1.0)
            rcp_t = epi_pool.tile([P, NJ], mybir.dt.float32)
            nc.vector.reciprocal(rcp_t[:], cnt_t[:])

            out_t = epi_pool.tile([P, NJ * HIDDEN], mybir.dt.float32)
            o3 = out_t[:].rearrange("p (j h) -> p j h", h=HIDDEN)
            acc_t = epi_pool.tile([P, NJ], mybir.dt.float32)
            for h in range(HIDDEN):
                nc.vector.tensor_scalar_mul(
                    acc_t[:], s3e[:, :, 0], w_t[:, h:h + 1])
                for f in range(1, IN_DIM):
                    nc.vector.scalar_tensor_tensor(
                        out=acc_t[:],
                        in0=s3e[:, :, f],
                        scalar=w_t[:, f * HIDDEN + h:f * HIDDEN + h + 1],
                        in1=acc_t[:],
                        op0=mybir.AluOpType.mult,
                        op1=mybir.AluOpType.add,
                    )
                nc.vector.tensor_tensor(
                    out=acc_t[:], in0=acc_t[:], in1=rcp_t[:],
                    op=mybir.AluOpType.mult)
                wr0 = IN_DIM * HIDDEN
                for f in range(IN_DIM):
                    nc.vector.scalar_tensor_tensor(
                        out=acc_t[:],
                        in0=x3[:, :, f],
                        scalar=w_t[:, wr0 + f * HIDDEN + h:wr0 + f * HIDDEN + h + 1],
                        in1=acc_t[:],
                        op0=mybir.AluOpType.mult,
                        op1=mybir.AluOpType.add,
                    )
                bl0 = 2 * IN_DIM * HIDDEN
                nc.vector.tensor_scalar_add(
                    o3[:, :, h], acc_t[:], w_t[:, bl0 + h:bl0 + h + 1])

            nc.sync.dma_start(
                out_d.ap().rearrange("(p j) h -> p (j h)", j=NJ), out_t[:])

    nc.compile()
    return nc


# ---------------------------------------------------------------- host
def _pack_bank(deg):
    """Assign nodes to 128 partition rows of RCOLS columns (scan chains
    across the bank's gather tiles, so each partition is one long row).

    Nodes are placed in output-column order (round j holds nodes p*NJ+j),
    LPT-balanced within each round (widest -> lightest row), columns
    packed densely.  This keeps round j's close positions near column
    16*j, allowing close-gathers to be scheduled a fixed few tiles behind
    the gather stream.  Returns (part, col0) per node.
    """
    w = np.maximum(deg, 1).astype(np.int64).reshape(128, NJ)
    loads = np.zeros(128, np.int64)
    part = np.empty((128, NJ), np.int64)
    col0 = np.empty((128, NJ), np.int64)
    for j in range(NJ):
        wj = w[:, j]
        o = np.argsort(-wj, kind="stable")        # widest first
        rows = np.argsort(loads, kind="stable")   # lightest row first
        part[o, j] = rows
        col0[o, j] = loads[rows]
        loads[rows] += wj[o]
    assert loads.max() <= RCOLS, f"row overflow: {loads.max()} > {RCOLS}"
    # close-tile bound consumed by the static close-emission schedule
    cl_tile = (col0 + w - 1) // COLS
    tmax = np.minimum(np.arange(NJ) // 4 + 2, TILES_PB - 1)
    assert (cl_tile <= tmax[None, :]).all(), (
        f"close tile exceeds schedule: {(cl_tile - tmax[None, :]).max()}")
    return part.reshape(-1), col0.reshape(-1)


def prepare_inputs(x, edge_index, W_l, b_l, W_r):
    x = np.asarray(x, np.float32)
    W_l = np.asarray(W_l, np.float32)
    b_l = np.asarray(b_l, np.float32)
    W_r = np.asarray(W_r, np.float32)
    src = np.asarray(edge_index[0], np.int64)
    dst = np.asarray(edge_index[1], np.int64)

    # bf16 feature table: 4 banks of 25604 rows (row 25600+ of each bank = 0)
    x_tab = np.zeros((XT_ROWS, ES), np.float32)
    for b in range(NBANKS):
        lo = b * BANK_N
        hi = min((b + 1) * BANK_N, N_NODES)
        if hi > lo:
            rows = b * BANK_STRIDE + np.arange(hi - lo)
            x_tab[rows, :IN_DIM] = x[lo:hi]
            x_tab[rows, IN_DIM] = 1.0
    x_tab = x_tab.astype(BF16)

    wcat = np.concatenate([W_l.reshape(-1), W_r.reshape(-1), b_l.reshape(-1)])
    wrep = np.ascontiguousarray(
        np.broadcast_to(wcat, (128, wcat.shape[0])), np.float32)

    core_of = dst // NODES_PC
    bank_of = src // BANK_N
    in_maps = []
    for c in range(N_CORES):
        sel_c = core_of == c
        src_c = src[sel_c]
        dst_c = dst[sel_c] - c * NODES_PC
        bank_c = bank_of[sel_c]

        idx_all = np.empty((NBANKS, TILES_PB, 128, IDX_PI // 16), np.int16)
        keep_all = np.zeros((NBANKS, SLOTS_PB), np.float32)
        cidx_all = np.empty((NBANKS, 128, NJ), np.int32)

        for b in range(NBANKS):
            sel_b = bank_c == b
            sb = (src_c[sel_b] - b * BANK_N).astype(np.int64)
            db = dst_c[sel_b]
            order = np.argsort(db, kind="stable")
            sb = sb[order]
            db = db[order]
            deg = np.bincount(db, minlength=NPAD)
            w = np.maximum(deg, 1)
            assert w.sum() <= SLOTS_PB, f"slots {w.sum()} exceed {SLOTS_PB}"
            part, col0 = _pack_bank(deg)

            # per-edge slot: global col -> (tile, col%COLS, partition)
            seg = np.zeros(NPAD + 1, np.int64)
            np.cumsum(deg, out=seg[1:])
            rank = np.arange(db.shape[0]) - seg[db]
            col = col0[db] + rank
            p = part[db]
            pos = (col // COLS) * IDX_PI + (col % COLS) * 128 + p

            stream = np.full(SLOTS_PB, ZROW, np.int64)
            stream[pos] = sb
            keep_all[b][pos[rank > 0]] = 1.0

            # close p_all row: ((b*T + tile)*128 + p)*COLS + col%COLS
            cl_col = col0 + w - 1
            cl_row = (((b * TILES_PB + cl_col // COLS) * 128 + part) * COLS
                      + cl_col % COLS)
            cidx_all[b] = cl_row.reshape(128, NJ).astype(np.int32)

            # wrap: instruction t gets stream[t*IDX_PI:(t+1)*IDX_PI];
            # position i -> partition i%16 (replicated x8), column i//16
            st = stream.reshape(TILES_PB, IDX_PI // 16, 16).astype(np.int16)
            wt = np.transpose(st, (0, 2, 1))          # [T, 16, IDX_PI//16]
            idx_all[b] = np.tile(wt, (1, 8, 1))

        keeps = keep_all.reshape(NBANKS, TILES_PB, COLS, 128).transpose(
            0, 1, 3, 2)  # [b, t, p, c]
        xsh = np.zeros((NPAD, 12), np.float32)
        xsh[:NODES_PC, :IN_DIM] = x[c * NODES_PC:(c + 1) * NODES_PC]
        in_maps.append({
            "x_t": x_tab,
            "idxs": idx_all,
            "keeps": np.ascontiguousarray(keeps).astype(BF16),
            "cidx": np.ascontiguousarray(
                cidx_all.transpose(1, 0, 2).reshape(128, NBANKS * NJ)),
            "xsh": xsh,
            "wrep": wrep,
        })
    return in_maps


# ---------------------------------------------------------------- runner
class SpmdRunner:
    def __init__(self, nc, n_cores):
        import jax
        from jax.sharding import Mesh, PartitionSpec
        from jax.experimental.shard_map import shard_map
        from concourse.bass2jax import (
            _bass_exec_p, install_neuronx_cc_hook, partition_id_tensor)

        install_neuronx_cc_hook()
        self.n_cores = n_cores
        pname = nc.partition_id_tensor.name if nc.partition_id_tensor else None
        in_names, out_names, out_avals, zero_outs = [], [], [], []
        for alloc in nc.m.functions[0].allocations:
            if not isinstance(alloc, mybir.MemoryLocationSet):
                continue
            name = alloc.memorylocations[0].name
            if alloc.kind == "ExternalInput":
                if name != pname:
                    in_names.append(name)
            elif alloc.kind == "ExternalOutput":
                out_names.append(name)
                shape = tuple(alloc.tensor_shape)
                dt_np = mybir.dt.np(alloc.dtype)
                out_avals.append(jax.core.ShapedArray(shape, dt_np))
                zero_outs.append(np.zeros(shape, dt_np))
        self.in_names, self.out_names = in_names, out_names
        self.zero_outs = zero_outs
        n_params, n_outs = len(in_names), len(out_names)
        all_names = in_names + out_names + ([pname] if pname else [])

        def _body(*args):
            operands = list(args)
            if pname is not None:
                operands.append(partition_id_tensor())
            return tuple(_bass_exec_p.bind(
                *operands, out_avals=tuple(out_avals),
                in_names=tuple(all_names), out_names=tuple(out_names),
                lowering_input_output_aliases=(),
                sim_require_finite=True, sim_require_nnan=True, nc=nc))

        devices = jax.devices()[:n_cores]
        mesh = Mesh(np.asarray(devices), ("core",))
        self._mesh = mesh
        specs_in = (PartitionSpec("core"),) * (n_params + n_outs)
        specs_out = (PartitionSpec("core"),) * n_outs
        self._fn = jax.jit(
            shard_map(_body, mesh=mesh, in_specs=specs_in,
                      out_specs=specs_out, check_rep=False),
            keep_unused=True)
        self._jax = jax

    def prepare(self, in_maps):
        per = [[np.asarray(m[n]) for n in self.in_names] for m in in_maps]
        cat = [np.concatenate([per[c][i] for c in range(self.n_cores)], axis=0)
               for i in range(len(self.in_names))]
        cat += [np.concatenate([z] * self.n_cores, axis=0)
                for z in self.zero_outs]
        return cat

    def device_put(self, args):
        """Ship prepared args to the devices once (for repeat timing)."""
        import jax
        from jax.sharding import NamedSharding, PartitionSpec
        mesh = self._mesh
        sh = NamedSharding(mesh, PartitionSpec("core"))
        out = [jax.device_put(a, sh) for a in args]
        jax.block_until_ready(out)
        return out

    def run(self, args):
        outs = self._fn(*args)
        self._jax.block_until_ready(outs)
        return outs

    def results(self, outs):
        res = [dict() for _ in range(self.n_cores)]
        for i, name in enumerate(self.out_names):
            for c, part in enumerate(
                    np.split(np.asarray(outs[i]), self.n_cores, axis=0)):
                res[c][name] = part
        return res


_CACHE = {}


def kernel(x, edge_index, W_l, b_l, W_r):
    if "runner" not in _CACHE:
        nc = build_program()
        _CACHE["runner"] = SpmdRunner(nc, N_CORES)
    runner = _CACHE["runner"]
    in_maps = prepare_inputs(x, edge_index, W_l, b_l, W_r)
    args = runner.prepare(in_maps)
    res = runner.results(runner.run(args))
    out = np.empty((N_NODES, HIDDEN), np.float32)
    for c in range(N_CORES):
        out[c * NODES_PC:(c + 1) * NODES_PC] = res[c]["out"][:NODES_PC]
    return out
